# revision 28
# baseline (speedup 1.0000x reference)
"""AssistedExcitation Trainium2 kernel (v2).

out[b,c,h,w] = x[b,c,h,w] + bbox_mask[b,h,w] * mean_c(x[b,:,h,w])

Data-parallel over 8 NeuronCores: 2 images per core, no collectives.
HBM-bandwidth bound: x ships and out returns as bfloat16 (bf16
quantization ~2e-3 rel-err vs the 2e-2 budget). 8-bit I/O was analyzed
and is a dead end on TRN2: no engine converts 8<->16-bit at >=2
elem/cycle/lane (DVE 2x_1P needs all-2-byte operands, ACT is 1
elem/cycle, PE+PSUM eviction is ACT-limited), so the expansion of an
8-bit x or compression of an 8-bit out always costs more than the DMA
bytes it saves.

v2 vs v1 (113.8us -> ~100us): the v1 trace showed a 10us DMA hole and
a degraded store tail, rooted in the on-device box raster (~12us
serial DVE chain) delaying the first store to 38us while loads filled
their pool and stalled. v2:
 - rasterizes per-box row/col 0-1 indicator tables on the HOST (tiny;
   validity & per-image selection folded into the row table). Device
   mask work is 3 matmuls + a clamp per image.
 - ALL constants ship as ONE DMA issued on the sync ring ahead of the
   x loads (separate const DMAs measured 6-9us late: their packets
   starve behind the x-load firehose at the shared SDMA engines).
 - PE warmup: ~14 dummy matmuls on a memset tile during the const
   window, so the HAM clock gate (1.2GHz cold / 2.4GHz warm, ~3.4us
   activity window) is released before the first real matmuls.
 - CHUNK 4096 (32 image rows), 8 uniform chunks. (Variable small-
   first/small-last chunking measured WORSE: +2 chunks of per-chunk
   backend cadence outweigh the earlier store start.)
 - mean via PE with a per-sub stationary (col 8s+j = 1/C iff j==s) so
   sub s's channel-mean lands on PSUM partition s: the mask*mean mul
   runs 8-lane on DVE.
 - corr broadcast via K=8 selector matmuls (bank-aligned PSUM writes;
   a matmul may not cross a PSUM bank); ACT evicts PSUM->bf16 SBUF so
   the output adds run 2x-mode all-bf16 on DVE.
 - queue placement measured: loads on sync ring, stores on scalar
   ring, mask-reshape DMAs on gpsimd. (Stores on gpsimd/SWDGE: bursty,
   +17us; stores on sync: block subsequent loads, +13us; mask DMAs on
   scalar: tiny descriptors stall the store ring, +12us.)
Remaining time is DMA-bound: ~34MB/core at the measured ~420GB/s
per-core fair share = ~81us + 7.2us fixed preamble + pipeline ends;
run-to-run HW variance is +/-6us.
"""

import sys

sys.path.insert(0, "/opt/trn_rl_repo")

import numpy as np
import ml_dtypes

import concourse.bacc as bacc
import concourse.bass as bass
import concourse.mybir as mybir
import concourse.tile as tile
from concourse import bass_utils

# Problem constants (hardcoded per harness contract)
B, C, H, W = 16, 256, 128, 128
N_BOX = 320
N_CORES = 8
B_SHARD = B // N_CORES  # 2 images per core
HW = H * W  # 16384
P = 128  # partitions
CHUNK = 4096  # max free-dim elements per x tile (32 rows of the image)
N_CHUNK = HW // CHUNK  # kept for test.py's copybench
SUB = 512  # max matmul moving free-dim (one PSUM bank of f32)
N_SUB = 8  # subs per chunk (mean lands on psum partitions 0..7)
NBOX_PAD = 384  # 320 boxes padded to 3 tiles of 128
N_BOX_TILES = NBOX_PAD // P  # 3
ALPHA = 1.0

# Uniform chunk schedule: 8 chunks of 32 rows. (A variable small-first/
# small-last schedule was tried and measured WORSE: the 2 extra chunks
# add ~11us of per-chunk backend cadence, more than the ~6us saved at
# the pipeline ends.)
CHUNK_ROWS = [
    (0, 0, 32), (0, 32, 32), (0, 64, 32), (0, 96, 32),
    (1, 0, 32), (1, 32, 32), (1, 64, 32), (1, 96, 32),
]  # (image, row0, nrows); nrows divisible by 8

F32 = mybir.dt.float32
BF16 = mybir.dt.bfloat16


def build_nc():
    """Build the per-core Bass graph (SPMD: same graph on all 8 cores)."""
    nc = bacc.Bacc(None, target_bir_lowering=False)

    x = nc.declare_dram_parameter("x", [B_SHARD, C, HW], BF16, isOutput=False)
    # all constants packed into ONE tensor so a single DMA (issued on the
    # sync ring AHEAD of the x loads) lands them before the load firehose
    # can starve them: cols 0:768 = rows indicators [n, j*384+t*128+h],
    # 768:1152 = cols indicators [n, t*128+w], 1152:1216 = wsum,
    # 1216:1728 = sel8 (rows 0:8).
    CW = B_SHARD * N_BOX_TILES * P + N_BOX_TILES * P + N_SUB * N_SUB + N_SUB * P
    consts_d = nc.declare_dram_parameter("consts", [P, CW], BF16, isOutput=False)
    out = nc.declare_dram_parameter("out", [B_SHARD, C, HW], BF16, isOutput=True)

    with tile.TileContext(nc) as tc:
        with (
            tc.tile_pool(name="const", bufs=1) as constp,
            tc.tile_pool(name="maskp", bufs=1) as maskp,
            tc.tile_pool(name="mfp", bufs=3) as mfp,
            tc.tile_pool(name="adp", bufs=3) as adp,
            tc.tile_pool(name="xp", bufs=5) as xp,
            tc.tile_pool(name="outp", bufs=3) as outp,
            tc.tile_pool(name="corrp", bufs=3) as corrp,
            tc.tile_pool(name="meanp", bufs=2, space=bass.MemorySpace.PSUM) as meanp,
            tc.tile_pool(name="pcp", bufs=3, space=bass.MemorySpace.PSUM) as pcp,
        ):
            # --- constants: ONE sync-ring DMA issued before any x load ---
            CW = B_SHARD * N_BOX_TILES * P + N_BOX_TILES * P + N_SUB * N_SUB + N_SUB * P
            cb = constp.tile([P, CW], BF16, tag="cb")
            nc.sync.dma_start(cb[:], consts_d[:])
            R0 = B_SHARD * N_BOX_TILES * P  # 768
            C0 = R0 + N_BOX_TILES * P  # 1152
            W0 = C0 + N_SUB * N_SUB  # 1216

            def rows_sl(j, t):
                o = (j * N_BOX_TILES + t) * P
                return cb[:, o : o + P]

            def cols_sl(t):
                return cb[:, R0 + t * P : R0 + (t + 1) * P]

            def wsum_sl(s):
                return cb[:, C0 + N_SUB * s : C0 + N_SUB * s + N_SUB]

            def sel8_sl(s):
                return cb[0:N_SUB, W0 + s * P : W0 + (s + 1) * P]

            # --- PE warmup: the HAM clock gate defaults to 1.2 GHz and only
            # releases after ~3.4us of sustained matmul activity. Burn dummy
            # matmuls on a memset scratch tile during the const-DMA window so
            # the first REAL matmuls run at 2.4 GHz. ---
            scratch = constp.tile([P, SUB], BF16, tag="scratch")
            nc.gpsimd.memset(scratch[:], 0)
            pwarm = meanp.tile([N_SUB, SUB], F32, tag="pmean")
            for w in range(14):
                nc.tensor.matmul(
                    pwarm[:], scratch[:, 0:N_SUB], scratch[:],
                    start=(w == 0), stop=(w == 13),
                )

            # --- per-chunk stage helpers (variable chunk sizes) ---
            st_loads = {}
            st_mean = {}
            st_ad = {}

            def frontL(k):
                """Issue the two x half-tile loads for chunk k (sync queue)."""
                b, row0, nrows = CHUNK_ROWS[k]
                ck = nrows * W
                csl = slice(row0 * W, row0 * W + ck)
                A = xp.tile([P, ck], BF16, tag="A", padded_shape=[P, CHUNK])
                nc.sync.dma_start(A[:], x[b, 0:P, csl])
                Bt = xp.tile([P, ck], BF16, tag="B", padded_shape=[P, CHUNK])
                nc.sync.dma_start(Bt[:], x[b, P:C, csl])
                st_loads[k] = (b, csl, A, Bt)

            def frontM(k):
                """Channel-mean matmuls for chunk k -> pmean psum [8,sub]."""
                _, _, A, Bt = st_loads[k]
                b, row0, nrows = CHUNK_ROWS[k]
                sub = nrows * W // N_SUB
                pmean = meanp.tile([N_SUB, sub], F32, tag="pmean",
                                   padded_shape=[N_SUB, SUB])
                for s in range(N_SUB):
                    ssl = slice(s * sub, (s + 1) * sub)
                    nc.tensor.matmul(
                        pmean[:], wsum_sl(s), A[:, ssl],
                        start=(s == 0), stop=False,
                    )
                    nc.tensor.matmul(
                        pmean[:], wsum_sl(s), Bt[:, ssl],
                        start=False, stop=(s == N_SUB - 1),
                    )
                st_mean[k] = pmean

            masks = []  # msb per image, filled by the mask prologue
            st_mf = {}

            def mfdma(k):
                """Mask reshape [nrows,128] -> [8,sub] (gpsimd SWDGE)."""
                b, row0, nrows = CHUNK_ROWS[k]
                sub = nrows * W // N_SUB
                mf = mfp.tile([N_SUB, sub], BF16, tag="mf", bufs=5,
                              padded_shape=[N_SUB, SUB])
                nc.gpsimd.dma_start(mf[:], masks[b][row0 : row0 + nrows, :])
                st_mf[k] = mf

            def frontB(k):
                """8-lane mask*mean -> ad (bf16 SBUF)."""
                b, row0, nrows = CHUNK_ROWS[k]
                sub = nrows * W // N_SUB
                pmean = st_mean[k]
                ad = adp.tile([N_SUB, sub], BF16, tag="ad",
                              padded_shape=[N_SUB, SUB])
                nc.vector.tensor_mul(ad[:], pmean[:], st_mf[k][:])
                st_ad[k] = ad

            def bcasts(k):
                """K=8 selector matmuls: corr quarters into psum (PE queue)."""
                b, row0, nrows = CHUNK_ROWS[k]
                sub = nrows * W // N_SUB
                ad = st_ad[k]
                pcs = []
                for q in range(4):
                    # each matmul output anchored at a PSUM bank boundary
                    # (offset 0 / SUB): a matmul may not cross a bank line
                    pc = pcp.tile([P, 2 * SUB], F32, tag="pc")
                    for h in range(2):
                        s = 2 * q + h
                        nc.tensor.matmul(
                            pc[:, h * SUB : h * SUB + sub],
                            sel8_sl(s), ad[:],
                            start=True, stop=True,
                        )
                    pcs.append(pc)
                return pcs

            def backend(k, pcs):
                """ACT evictions, bf16 adds, stores for chunk k."""
                b, csl, A, Bt = st_loads[k]
                _, row0, nrows = CHUNK_ROWS[k]
                ck = nrows * W
                sub = ck // N_SUB
                corr = corrp.tile([P, ck], BF16, tag="corr",
                                  padded_shape=[P, CHUNK])
                for q in range(4):
                    if sub == SUB:
                        nc.scalar.activation(
                            corr[:, q * 2 * sub : (q + 1) * 2 * sub],
                            pcs[q][:], mybir.ActivationFunctionType.Copy,
                        )
                    else:
                        for h in range(2):
                            nc.scalar.activation(
                                corr[:, (2 * q + h) * sub : (2 * q + h + 1) * sub],
                                pcs[q][:, h * SUB : h * SUB + sub],
                                mybir.ActivationFunctionType.Copy,
                            )
                half = ck // 2
                oA = outp.tile([P, ck], BF16, tag="oA", padded_shape=[P, CHUNK])
                oB = outp.tile([P, ck], BF16, tag="oB", padded_shape=[P, CHUNK])
                nc.vector.tensor_add(oA[:, 0:half], A[:, 0:half], corr[:, 0:half])
                nc.vector.tensor_add(oA[:, half:ck], A[:, half:ck], corr[:, half:ck])
                nc.scalar.dma_start(out[b, 0:P, csl], oA[:])
                nc.vector.tensor_add(oB[:, 0:half], Bt[:, 0:half], corr[:, 0:half])
                nc.vector.tensor_add(oB[:, half:ck], Bt[:, half:ck], corr[:, half:ck])
                nc.scalar.dma_start(out[b, P:C, csl], oB[:])

            # --- prologue: first chunks' loads right after the const DMA,
            # so the DMA stream starts the moment the preamble ends ---
            frontL(0)
            frontL(1)
            frontL(2)
            frontL(3)

            # --- per-image mask: psum[h,w] = sum_n rows[n,h]*cols[n,w]; clamp.
            # rows already carries validity & per-image selection (host). ---
            T3 = N_BOX_TILES
            for j in range(B_SHARD):
                pm = pcp.tile([P, 2 * SUB], F32, tag="pc")
                for t in range(T3):
                    nc.tensor.matmul(
                        pm[:, 0:W],
                        rows_sl(j, t),
                        cols_sl(t),
                        start=(t == 0), stop=(t == T3 - 1),
                    )
                msb = maskp.tile([P, W], BF16, tag=f"msb{j}")
                nc.vector.tensor_scalar_min(msb[:], pm[:, 0:W], 1.0)
                masks.append(msb)

            # --- software-pipelined main loop over the chunk schedule ---
            # (emission order = per-engine queue order: bcasts/backend of k
            # BEFORE next chunk's means/mul, so the store path never queues
            # behind future work)
            for k in range(4):
                mfdma(k)
            frontM(0)
            frontB(0)
            N = len(CHUNK_ROWS)
            for k in range(N):
                if k + 4 < N:
                    frontL(k + 4)
                    mfdma(k + 4)
                if k + 1 < N:
                    frontM(k + 1)
                    frontB(k + 1)
                pcs = bcasts(k)
                backend(k, pcs)

    return nc


def _host_prep(x, bboxes, batch_idx):
    """Shard x; rasterize per-box row/col indicator tables (tiny host prep,
    mirrors the reference's int-trunc/clamp semantics exactly)."""
    x = np.ascontiguousarray(np.asarray(x, dtype=np.float32)).reshape(B, C, HW)
    x16 = x.astype(ml_dtypes.bfloat16)
    bb = np.asarray(bboxes, dtype=np.float32)
    bidx = np.asarray(batch_idx).astype(np.int64)

    xc, yc, bw, bh = bb[:, 0], bb[:, 1], bb[:, 2], bb[:, 3]
    x1 = np.maximum(0, np.trunc((xc - bw / 2) * W)).astype(np.int32)
    y1 = np.maximum(0, np.trunc((yc - bh / 2) * H)).astype(np.int32)
    x2 = np.minimum(W - 1, np.trunc((xc + bw / 2) * W)).astype(np.int32)
    y2 = np.minimum(H - 1, np.trunc((yc + bh / 2) * H)).astype(np.int32)
    valid = (x2 > x1) & (y2 > y1)

    hh = np.arange(H, dtype=np.int32)
    rows_ind = ((hh[None, :] >= y1[:, None]) & (hh[None, :] <= y2[:, None]))
    ww = np.arange(W, dtype=np.int32)
    cols_ind = ((ww[None, :] >= x1[:, None]) & (ww[None, :] <= x2[:, None]))

    R0 = B_SHARD * N_BOX_TILES * P  # 768
    C0 = R0 + N_BOX_TILES * P  # 1152
    W0 = C0 + N_SUB * N_SUB  # 1216
    CW = W0 + N_SUB * P  # 1728

    base = np.zeros((P, CW), dtype=ml_dtypes.bfloat16)
    # cols[n, t*128 + w]: box n = t*128 + p lives on partition p of tile t
    for t in range(N_BOX_TILES):
        lo, hi = t * P, min((t + 1) * P, N_BOX)
        base[0 : hi - lo, R0 + t * P : R0 + t * P + W] = cols_ind[lo:hi]
    # wsum: col 8s+j = ALPHA/C iff j==s  (mean lands on psum partition s)
    for s in range(N_SUB):
        base[:, C0 + N_SUB * s + s] = ALPHA / C
    # sel8 slice s = [8,128] stationary whose row s is all-ones
    for s in range(N_SUB):
        base[s, W0 + s * P : W0 + (s + 1) * P] = 1.0

    in_maps = []
    for i in range(N_CORES):
        cb = base.copy()
        for j in range(B_SHARD):
            on = valid & (bidx == (i * B_SHARD + j))
            for t in range(N_BOX_TILES):
                lo, hi = t * P, min((t + 1) * P, N_BOX)
                blk = rows_ind[lo:hi] * on[lo:hi, None]
                cb[0 : hi - lo, (j * N_BOX_TILES + t) * P : (j * N_BOX_TILES + t) * P + H] = blk
        in_maps.append(
            {
                "x": np.ascontiguousarray(x16[i * B_SHARD : (i + 1) * B_SHARD]),
                "consts": cb,
            }
        )
    return in_maps


def kernel(x, bboxes, batch_idx):
    in_maps = _host_prep(x, bboxes, batch_idx)
    nc = build_nc()
    nc.finalize()
    res = bass_utils.run_bass_kernel_spmd(nc, in_maps, core_ids=list(range(N_CORES)))
    shards = [res.results[i]["out"] for i in range(N_CORES)]
    return (
        np.concatenate(shards, axis=0).astype(np.float32).reshape(B, C, H, W)
    )


if __name__ == "__main__":
    nc = build_nc()
    nc.finalize()
    print("built ok:", len(nc.inst_map), "instructions")


# revision 32
# speedup vs baseline: 1.0208x; 1.0208x over previous
"""AssistedExcitation Trainium2 kernel (v2).

out[b,c,h,w] = x[b,c,h,w] + bbox_mask[b,h,w] * mean_c(x[b,:,h,w])

Data-parallel over 8 NeuronCores: 2 images per core, no collectives.
HBM-bandwidth bound: x ships and out returns as bfloat16 (bf16
quantization ~2e-3 rel-err vs the 2e-2 budget). 8-bit I/O was analyzed
and is a dead end on TRN2: no engine converts 8<->16-bit at >=2
elem/cycle/lane (DVE 2x_1P needs all-2-byte operands, ACT is 1
elem/cycle, PE+PSUM eviction is ACT-limited), so the expansion of an
8-bit x or compression of an 8-bit out always costs more than the DMA
bytes it saves.

v2 vs v1 (113.8us -> ~100us): the v1 trace showed a 10us DMA hole and
a degraded store tail, rooted in the on-device box raster (~12us
serial DVE chain) delaying the first store to 38us while loads filled
their pool and stalled. v2:
 - rasterizes per-box row/col 0-1 indicator tables on the HOST (tiny;
   validity & per-image selection folded into the row table). Device
   mask work is 3 matmuls + a clamp per image.
 - ALL constants ship as ONE DMA issued on the sync ring ahead of the
   x loads (separate const DMAs measured 6-9us late: their packets
   starve behind the x-load firehose at the shared SDMA engines).
 - PE warmup: ~14 dummy matmuls on a memset tile during the const
   window, so the HAM clock gate (1.2GHz cold / 2.4GHz warm, ~3.4us
   activity window) is released before the first real matmuls.
 - CHUNK 4096 (32 image rows), 8 uniform chunks. (Variable small-
   first/small-last chunking measured WORSE: +2 chunks of per-chunk
   backend cadence outweigh the earlier store start.)
 - mean via PE with a per-sub stationary (col 8s+j = 1/C iff j==s) so
   sub s's channel-mean lands on PSUM partition s: the mask*mean mul
   runs 8-lane on DVE.
 - corr broadcast via K=8 selector matmuls (bank-aligned PSUM writes;
   a matmul may not cross a PSUM bank); ACT evicts PSUM->bf16 SBUF so
   the output adds run 2x-mode all-bf16 on DVE.
 - queue placement measured: loads on sync ring, stores on scalar
   ring, mask-reshape DMAs on gpsimd. (Stores on gpsimd/SWDGE: bursty,
   +17us; stores on sync: block subsequent loads, +13us; mask DMAs on
   scalar: tiny descriptors stall the store ring, +12us.)
Remaining time is DMA-bound: ~34MB/core at the measured ~420GB/s
per-core fair share = ~81us + 7.2us fixed preamble + pipeline ends;
run-to-run HW variance is +/-6us.
"""

import sys

sys.path.insert(0, "/opt/trn_rl_repo")

import numpy as np
import ml_dtypes

import concourse.bacc as bacc
import concourse.bass as bass
import concourse.mybir as mybir
import concourse.tile as tile
from concourse import bass_utils

# Problem constants (hardcoded per harness contract)
B, C, H, W = 16, 256, 128, 128
N_BOX = 320
N_CORES = 8
B_SHARD = B // N_CORES  # 2 images per core
HW = H * W  # 16384
P = 128  # partitions
CHUNK = 4096  # max free-dim elements per x tile (32 rows of the image)
N_CHUNK = HW // CHUNK  # kept for test.py's copybench
SUB = 512  # max matmul moving free-dim (one PSUM bank of f32)
N_SUB = 8  # subs per chunk (mean lands on psum partitions 0..7)
NBOX_PAD = 384  # 320 boxes padded to 3 tiles of 128
N_BOX_TILES = NBOX_PAD // P  # 3
ALPHA = 1.0

# Uniform chunk schedule: 8 chunks of 32 rows. (A variable small-first/
# small-last schedule was tried and measured WORSE: the 2 extra chunks
# add ~11us of per-chunk backend cadence, more than the ~6us saved at
# the pipeline ends.)
CHUNK_ROWS = [
    (0, 0, 32), (0, 32, 32), (0, 64, 32), (0, 96, 32),
    (1, 0, 32), (1, 32, 32), (1, 64, 32), (1, 96, 32),
]  # (image, row0, nrows); nrows divisible by 8

F32 = mybir.dt.float32
BF16 = mybir.dt.bfloat16


def build_nc():
    """Build the per-core Bass graph (SPMD: same graph on all 8 cores)."""
    nc = bacc.Bacc(None, target_bir_lowering=False)

    x = nc.declare_dram_parameter("x", [B_SHARD, C, HW], BF16, isOutput=False)
    # all constants packed into ONE tensor so a single DMA (issued on the
    # sync ring AHEAD of the x loads) lands them before the load firehose
    # can starve them: cols 0:768 = rows indicators [n, j*384+t*128+h],
    # 768:1152 = cols indicators [n, t*128+w], 1152:1216 = wsum,
    # 1216:1728 = sel8 (rows 0:8).
    CW = B_SHARD * N_BOX_TILES * P + N_BOX_TILES * P + N_SUB * N_SUB + N_SUB * P
    consts_d = nc.declare_dram_parameter("consts", [P, CW], BF16, isOutput=False)
    out = nc.declare_dram_parameter("out", [B_SHARD, C, HW], BF16, isOutput=True)

    with tile.TileContext(nc) as tc:
        with (
            tc.tile_pool(name="const", bufs=1) as constp,
            tc.tile_pool(name="maskp", bufs=1) as maskp,
            tc.tile_pool(name="mfp", bufs=3) as mfp,
            tc.tile_pool(name="adp", bufs=3) as adp,
            tc.tile_pool(name="xp", bufs=6) as xp,
            tc.tile_pool(name="outp", bufs=3) as outp,
            tc.tile_pool(name="corrp", bufs=3) as corrp,
            tc.tile_pool(name="meanp", bufs=2, space=bass.MemorySpace.PSUM) as meanp,
            tc.tile_pool(name="pcp", bufs=3, space=bass.MemorySpace.PSUM) as pcp,
        ):
            # --- constants: ONE sync-ring DMA issued before any x load ---
            CW = B_SHARD * N_BOX_TILES * P + N_BOX_TILES * P + N_SUB * N_SUB + N_SUB * P
            cb = constp.tile([P, CW], BF16, tag="cb")
            nc.sync.dma_start(cb[:], consts_d[:])
            R0 = B_SHARD * N_BOX_TILES * P  # 768
            C0 = R0 + N_BOX_TILES * P  # 1152
            W0 = C0 + N_SUB * N_SUB  # 1216

            def rows_sl(j, t):
                o = (j * N_BOX_TILES + t) * P
                return cb[:, o : o + P]

            def cols_sl(t):
                return cb[:, R0 + t * P : R0 + (t + 1) * P]

            def wsum_sl(s):
                return cb[:, C0 + N_SUB * s : C0 + N_SUB * s + N_SUB]

            def sel8_sl(s):
                return cb[0:N_SUB, W0 + s * P : W0 + (s + 1) * P]

            # --- PE warmup: the HAM clock gate defaults to 1.2 GHz and only
            # releases after ~3.4us of sustained matmul activity. Burn dummy
            # matmuls on a memset scratch tile during the const-DMA window so
            # the first REAL matmuls run at 2.4 GHz. ---
            scratch = constp.tile([P, SUB], BF16, tag="scratch")
            nc.gpsimd.memset(scratch[:], 0)
            pwarm = meanp.tile([N_SUB, SUB], F32, tag="pmean")
            for w in range(14):
                nc.tensor.matmul(
                    pwarm[:], scratch[:, 0:N_SUB], scratch[:],
                    start=(w == 0), stop=(w == 13),
                )

            # --- per-chunk stage helpers (variable chunk sizes) ---
            st_loads = {}
            st_mean = {}
            st_ad = {}

            def frontL(k):
                """Issue the two x half-tile loads for chunk k (sync queue)."""
                b, row0, nrows = CHUNK_ROWS[k]
                ck = nrows * W
                csl = slice(row0 * W, row0 * W + ck)
                A = xp.tile([P, ck], BF16, tag="A", padded_shape=[P, CHUNK])
                nc.sync.dma_start(A[:], x[b, 0:P, csl])
                Bt = xp.tile([P, ck], BF16, tag="B", padded_shape=[P, CHUNK])
                nc.sync.dma_start(Bt[:], x[b, P:C, csl])
                st_loads[k] = (b, csl, A, Bt)

            def frontM(k):
                """Channel-mean matmuls for chunk k -> pmean psum [8,sub]."""
                _, _, A, Bt = st_loads[k]
                b, row0, nrows = CHUNK_ROWS[k]
                sub = nrows * W // N_SUB
                pmean = meanp.tile([N_SUB, sub], F32, tag="pmean",
                                   padded_shape=[N_SUB, SUB])
                # all A-half matmuls first: they overlap the B-half's DMA
                # transfer, so the mean completes ~1.7us after B lands
                # instead of ~3.5us
                for s in range(N_SUB):
                    ssl = slice(s * sub, (s + 1) * sub)
                    nc.tensor.matmul(
                        pmean[:], wsum_sl(s), A[:, ssl],
                        start=(s == 0), stop=False,
                    )
                for s in range(N_SUB):
                    ssl = slice(s * sub, (s + 1) * sub)
                    nc.tensor.matmul(
                        pmean[:], wsum_sl(s), Bt[:, ssl],
                        start=False, stop=(s == N_SUB - 1),
                    )
                st_mean[k] = pmean

            masks = []  # msb per image, filled by the mask prologue
            st_mf = {}

            def mfdma(k):
                """Mask reshape [nrows,128] -> [8,sub] (gpsimd SWDGE)."""
                b, row0, nrows = CHUNK_ROWS[k]
                sub = nrows * W // N_SUB
                mf = mfp.tile([N_SUB, sub], BF16, tag="mf", bufs=5,
                              padded_shape=[N_SUB, SUB])
                nc.gpsimd.dma_start(mf[:], masks[b][row0 : row0 + nrows, :])
                st_mf[k] = mf

            def frontB(k):
                """8-lane mask*mean -> ad (bf16 SBUF)."""
                b, row0, nrows = CHUNK_ROWS[k]
                sub = nrows * W // N_SUB
                pmean = st_mean[k]
                ad = adp.tile([N_SUB, sub], BF16, tag="ad",
                              padded_shape=[N_SUB, SUB])
                nc.vector.tensor_mul(ad[:], pmean[:], st_mf[k][:])
                st_ad[k] = ad

            def bcasts(k):
                """K=8 selector matmuls: corr quarters into psum (PE queue)."""
                b, row0, nrows = CHUNK_ROWS[k]
                sub = nrows * W // N_SUB
                ad = st_ad[k]
                pcs = []
                for q in range(4):
                    # each matmul output anchored at a PSUM bank boundary
                    # (offset 0 / SUB): a matmul may not cross a bank line
                    pc = pcp.tile([P, 2 * SUB], F32, tag="pc")
                    for h in range(2):
                        s = 2 * q + h
                        nc.tensor.matmul(
                            pc[:, h * SUB : h * SUB + sub],
                            sel8_sl(s), ad[:],
                            start=True, stop=True,
                        )
                    pcs.append(pc)
                return pcs

            def backend(k, pcs):
                """ACT evictions, bf16 adds, stores for chunk k."""
                b, csl, A, Bt = st_loads[k]
                _, row0, nrows = CHUNK_ROWS[k]
                ck = nrows * W
                sub = ck // N_SUB
                corr = corrp.tile([P, ck], BF16, tag="corr",
                                  padded_shape=[P, CHUNK])
                for q in range(4):
                    if sub == SUB:
                        nc.scalar.activation(
                            corr[:, q * 2 * sub : (q + 1) * 2 * sub],
                            pcs[q][:], mybir.ActivationFunctionType.Copy,
                        )
                    else:
                        for h in range(2):
                            nc.scalar.activation(
                                corr[:, (2 * q + h) * sub : (2 * q + h + 1) * sub],
                                pcs[q][:, h * SUB : h * SUB + sub],
                                mybir.ActivationFunctionType.Copy,
                            )
                half = ck // 2
                oA = outp.tile([P, ck], BF16, tag="oA", padded_shape=[P, CHUNK])
                oB = outp.tile([P, ck], BF16, tag="oB", padded_shape=[P, CHUNK])
                nc.vector.tensor_add(oA[:, 0:half], A[:, 0:half], corr[:, 0:half])
                nc.vector.tensor_add(oA[:, half:ck], A[:, half:ck], corr[:, half:ck])
                nc.scalar.dma_start(out[b, 0:P, csl], oA[:])
                nc.vector.tensor_add(oB[:, 0:half], Bt[:, 0:half], corr[:, 0:half])
                nc.vector.tensor_add(oB[:, half:ck], Bt[:, half:ck], corr[:, half:ck])
                nc.scalar.dma_start(out[b, P:C, csl], oB[:])

            # --- prologue: first chunks' loads right after the const DMA,
            # so the DMA stream starts the moment the preamble ends ---
            frontL(0)
            frontL(1)
            frontL(2)
            frontL(3)
            frontL(4)

            # --- per-image mask: psum[h,w] = sum_n rows[n,h]*cols[n,w]; clamp.
            # rows already carries validity & per-image selection (host). ---
            T3 = N_BOX_TILES
            for j in range(B_SHARD):
                pm = pcp.tile([P, 2 * SUB], F32, tag="pc")
                for t in range(T3):
                    nc.tensor.matmul(
                        pm[:, 0:W],
                        rows_sl(j, t),
                        cols_sl(t),
                        start=(t == 0), stop=(t == T3 - 1),
                    )
                msb = maskp.tile([P, W], BF16, tag=f"msb{j}")
                nc.vector.tensor_scalar_min(msb[:], pm[:, 0:W], 1.0)
                masks.append(msb)

            # --- software-pipelined main loop over the chunk schedule ---
            # (emission order = per-engine queue order: bcasts/backend of k
            # BEFORE next chunk's means/mul, so the store path never queues
            # behind future work)
            for k in range(4):
                mfdma(k)
            frontM(0)
            frontB(0)
            N = len(CHUNK_ROWS)
            for k in range(N):
                if k + 5 < N:
                    frontL(k + 5)
                if k + 4 < N:
                    mfdma(k + 4)
                if k + 1 < N:
                    frontM(k + 1)
                    frontB(k + 1)
                pcs = bcasts(k)
                backend(k, pcs)

    return nc


def _host_prep(x, bboxes, batch_idx):
    """Shard x; rasterize per-box row/col indicator tables (tiny host prep,
    mirrors the reference's int-trunc/clamp semantics exactly)."""
    x = np.ascontiguousarray(np.asarray(x, dtype=np.float32)).reshape(B, C, HW)
    x16 = x.astype(ml_dtypes.bfloat16)
    bb = np.asarray(bboxes, dtype=np.float32)
    bidx = np.asarray(batch_idx).astype(np.int64)

    xc, yc, bw, bh = bb[:, 0], bb[:, 1], bb[:, 2], bb[:, 3]
    x1 = np.maximum(0, np.trunc((xc - bw / 2) * W)).astype(np.int32)
    y1 = np.maximum(0, np.trunc((yc - bh / 2) * H)).astype(np.int32)
    x2 = np.minimum(W - 1, np.trunc((xc + bw / 2) * W)).astype(np.int32)
    y2 = np.minimum(H - 1, np.trunc((yc + bh / 2) * H)).astype(np.int32)
    valid = (x2 > x1) & (y2 > y1)

    hh = np.arange(H, dtype=np.int32)
    rows_ind = ((hh[None, :] >= y1[:, None]) & (hh[None, :] <= y2[:, None]))
    ww = np.arange(W, dtype=np.int32)
    cols_ind = ((ww[None, :] >= x1[:, None]) & (ww[None, :] <= x2[:, None]))

    R0 = B_SHARD * N_BOX_TILES * P  # 768
    C0 = R0 + N_BOX_TILES * P  # 1152
    W0 = C0 + N_SUB * N_SUB  # 1216
    CW = W0 + N_SUB * P  # 1728

    base = np.zeros((P, CW), dtype=ml_dtypes.bfloat16)
    # cols[n, t*128 + w]: box n = t*128 + p lives on partition p of tile t
    for t in range(N_BOX_TILES):
        lo, hi = t * P, min((t + 1) * P, N_BOX)
        base[0 : hi - lo, R0 + t * P : R0 + t * P + W] = cols_ind[lo:hi]
    # wsum: col 8s+j = ALPHA/C iff j==s  (mean lands on psum partition s)
    for s in range(N_SUB):
        base[:, C0 + N_SUB * s + s] = ALPHA / C
    # sel8 slice s = [8,128] stationary whose row s is all-ones
    for s in range(N_SUB):
        base[s, W0 + s * P : W0 + (s + 1) * P] = 1.0

    in_maps = []
    for i in range(N_CORES):
        cb = base.copy()
        for j in range(B_SHARD):
            on = valid & (bidx == (i * B_SHARD + j))
            for t in range(N_BOX_TILES):
                lo, hi = t * P, min((t + 1) * P, N_BOX)
                blk = rows_ind[lo:hi] * on[lo:hi, None]
                cb[0 : hi - lo, (j * N_BOX_TILES + t) * P : (j * N_BOX_TILES + t) * P + H] = blk
        in_maps.append(
            {
                "x": np.ascontiguousarray(x16[i * B_SHARD : (i + 1) * B_SHARD]),
                "consts": cb,
            }
        )
    return in_maps


def kernel(x, bboxes, batch_idx):
    in_maps = _host_prep(x, bboxes, batch_idx)
    nc = build_nc()
    nc.finalize()
    res = bass_utils.run_bass_kernel_spmd(nc, in_maps, core_ids=list(range(N_CORES)))
    shards = [res.results[i]["out"] for i in range(N_CORES)]
    return (
        np.concatenate(shards, axis=0).astype(np.float32).reshape(B, C, H, W)
    )


if __name__ == "__main__":
    nc = build_nc()
    nc.finalize()
    print("built ok:", len(nc.inst_map), "instructions")


# revision 33
# speedup vs baseline: 1.0251x; 1.0042x over previous
"""AssistedExcitation Trainium2 kernel (v2).

out[b,c,h,w] = x[b,c,h,w] + bbox_mask[b,h,w] * mean_c(x[b,:,h,w])

Data-parallel over 8 NeuronCores: 2 images per core, no collectives.
HBM-bandwidth bound: x ships and out returns as bfloat16 (bf16
quantization ~2e-3 rel-err vs the 2e-2 budget). 8-bit I/O was analyzed
and is a dead end on TRN2: no engine converts 8<->16-bit at >=2
elem/cycle/lane (DVE 2x_1P needs all-2-byte operands, ACT is 1
elem/cycle, PE+PSUM eviction is ACT-limited), so the expansion of an
8-bit x or compression of an 8-bit out always costs more than the DMA
bytes it saves.

v2 vs v1 (113.8us -> ~100us): the v1 trace showed a 10us DMA hole and
a degraded store tail, rooted in the on-device box raster (~12us
serial DVE chain) delaying the first store to 38us while loads filled
their pool and stalled. v2:
 - rasterizes per-box row/col 0-1 indicator tables on the HOST (tiny;
   validity & per-image selection folded into the row table). Device
   mask work is 3 matmuls + a clamp per image.
 - ALL constants ship as ONE DMA issued on the sync ring ahead of the
   x loads (separate const DMAs measured 6-9us late: their packets
   starve behind the x-load firehose at the shared SDMA engines).
 - PE warmup: ~14 dummy matmuls on a memset tile during the const
   window, so the HAM clock gate (1.2GHz cold / 2.4GHz warm, ~3.4us
   activity window) is released before the first real matmuls.
 - CHUNK 4096 (32 image rows), 8 uniform chunks. (Variable small-
   first/small-last chunking measured WORSE: +2 chunks of per-chunk
   backend cadence outweigh the earlier store start.)
 - mean via PE with a per-sub stationary (col 8s+j = 1/C iff j==s) so
   sub s's channel-mean lands on PSUM partition s: the mask*mean mul
   runs 8-lane on DVE. All A-half matmuls are emitted before the
   B-half's so they overlap the B half's DMA transfer.
 - corr broadcast via K=8 selector matmuls (bank-aligned PSUM writes;
   a matmul may not cross a PSUM bank); ACT evicts PSUM->bf16 SBUF so
   the output adds run 2x-mode all-bf16 on DVE.
 - queue placement measured: loads on sync ring, stores on scalar
   ring, mask-reshape DMAs on gpsimd. (Stores on gpsimd/SWDGE: bursty,
   +17us; stores on sync: block subsequent loads, +13us; mask DMAs on
   scalar: tiny descriptors stall the store ring, +12us.)
Remaining time is DMA-bound: ~34MB/core at the measured ~420GB/s
per-core fair share = ~81us + 7.2us fixed preamble + pipeline ends;
run-to-run HW variance is +/-6us.
"""

import sys

sys.path.insert(0, "/opt/trn_rl_repo")

import numpy as np
import ml_dtypes

import concourse.bacc as bacc
import concourse.bass as bass
import concourse.mybir as mybir
import concourse.tile as tile
from concourse import bass_utils

# Problem constants (hardcoded per harness contract)
B, C, H, W = 16, 256, 128, 128
N_BOX = 320
N_CORES = 8
B_SHARD = B // N_CORES  # 2 images per core
HW = H * W  # 16384
P = 128  # partitions
CHUNK = 4096  # max free-dim elements per x tile (32 rows of the image)
N_CHUNK = HW // CHUNK  # kept for test.py's copybench
SUB = 512  # max matmul moving free-dim (one PSUM bank of f32)
N_SUB = 8  # subs per chunk (mean lands on psum partitions 0..7)
NBOX_PAD = 384  # 320 boxes padded to 3 tiles of 128
N_BOX_TILES = NBOX_PAD // P  # 3
ALPHA = 1.0

# Uniform chunk schedule: 8 chunks of 32 rows. (A variable small-first/
# small-last schedule was tried and measured WORSE: the 2 extra chunks
# add ~11us of per-chunk backend cadence, more than the ~6us saved at
# the pipeline ends.)
CHUNK_ROWS = [
    (0, 0, 32), (0, 32, 32), (0, 64, 32), (0, 96, 32),
    (1, 0, 32), (1, 32, 32), (1, 64, 32), (1, 96, 32),
]  # (image, row0, nrows); nrows divisible by 8

F32 = mybir.dt.float32
BF16 = mybir.dt.bfloat16


def build_nc():
    """Build the per-core Bass graph (SPMD: same graph on all 8 cores)."""
    nc = bacc.Bacc(None, target_bir_lowering=False)

    x = nc.declare_dram_parameter("x", [B_SHARD, C, HW], BF16, isOutput=False)
    # all constants packed into ONE tensor so a single DMA (issued on the
    # sync ring AHEAD of the x loads) lands them before the load firehose
    # can starve them: cols 0:768 = rows indicators [n, j*384+t*128+h],
    # 768:1152 = cols indicators [n, t*128+w], 1152:1216 = wsum,
    # 1216:1728 = sel8 (rows 0:8).
    CW = B_SHARD * N_BOX_TILES * P + N_BOX_TILES * P + N_SUB * N_SUB + N_SUB * P
    consts_d = nc.declare_dram_parameter("consts", [P, CW], BF16, isOutput=False)
    out = nc.declare_dram_parameter("out", [B_SHARD, C, HW], BF16, isOutput=True)

    with tile.TileContext(nc) as tc:
        with (
            tc.tile_pool(name="const", bufs=1) as constp,
            tc.tile_pool(name="maskp", bufs=1) as maskp,
            tc.tile_pool(name="mfp", bufs=3) as mfp,
            tc.tile_pool(name="adp", bufs=3) as adp,
            tc.tile_pool(name="xp", bufs=6) as xp,
            tc.tile_pool(name="outp", bufs=3) as outp,
            tc.tile_pool(name="corrp", bufs=3) as corrp,
            tc.tile_pool(name="meanp", bufs=2, space=bass.MemorySpace.PSUM) as meanp,
            tc.tile_pool(name="pcp", bufs=3, space=bass.MemorySpace.PSUM) as pcp,
        ):
            # --- constants: ONE sync-ring DMA issued before any x load ---
            CW = B_SHARD * N_BOX_TILES * P + N_BOX_TILES * P + N_SUB * N_SUB + N_SUB * P
            cb = constp.tile([P, CW], BF16, tag="cb")
            nc.sync.dma_start(cb[:], consts_d[:])
            R0 = B_SHARD * N_BOX_TILES * P  # 768
            C0 = R0 + N_BOX_TILES * P  # 1152
            W0 = C0 + N_SUB * N_SUB  # 1216

            def rows_sl(j, t):
                o = (j * N_BOX_TILES + t) * P
                return cb[:, o : o + P]

            def cols_sl(t):
                return cb[:, R0 + t * P : R0 + (t + 1) * P]

            def wsum_sl(s):
                return cb[:, C0 + N_SUB * s : C0 + N_SUB * s + N_SUB]

            def sel8_sl(s):
                return cb[0:N_SUB, W0 + s * P : W0 + (s + 1) * P]

            # --- PE warmup: the HAM clock gate defaults to 1.2 GHz and only
            # releases after ~3.4us of sustained matmul activity. Burn dummy
            # matmuls on a memset scratch tile during the const-DMA window so
            # the first REAL matmuls run at 2.4 GHz. ---
            scratch = constp.tile([P, SUB], BF16, tag="scratch")
            nc.gpsimd.memset(scratch[:], 0)
            pwarm = meanp.tile([N_SUB, SUB], F32, tag="pmean")
            for w in range(14):
                nc.tensor.matmul(
                    pwarm[:], scratch[:, 0:N_SUB], scratch[:],
                    start=(w == 0), stop=(w == 13),
                )

            # --- per-chunk stage helpers (variable chunk sizes) ---
            st_loads = {}
            st_mean = {}
            st_ad = {}

            def frontL(k):
                """Issue the two x half-tile loads for chunk k (sync queue)."""
                b, row0, nrows = CHUNK_ROWS[k]
                ck = nrows * W
                csl = slice(row0 * W, row0 * W + ck)
                A = xp.tile([P, ck], BF16, tag="A", padded_shape=[P, CHUNK])
                nc.sync.dma_start(A[:], x[b, 0:P, csl])
                Bt = xp.tile([P, ck], BF16, tag="B", padded_shape=[P, CHUNK])
                nc.sync.dma_start(Bt[:], x[b, P:C, csl])
                st_loads[k] = (b, csl, A, Bt)

            def frontM(k):
                """Channel-mean matmuls for chunk k -> pmean psum [8,sub]."""
                _, _, A, Bt = st_loads[k]
                b, row0, nrows = CHUNK_ROWS[k]
                sub = nrows * W // N_SUB
                pmean = meanp.tile([N_SUB, sub], F32, tag="pmean",
                                   padded_shape=[N_SUB, SUB])
                # all A-half matmuls first: they overlap the B-half's DMA
                # transfer, so the mean completes ~1.7us after B lands
                # instead of ~3.5us
                for s in range(N_SUB):
                    ssl = slice(s * sub, (s + 1) * sub)
                    nc.tensor.matmul(
                        pmean[:], wsum_sl(s), A[:, ssl],
                        start=(s == 0), stop=False,
                    )
                for s in range(N_SUB):
                    ssl = slice(s * sub, (s + 1) * sub)
                    nc.tensor.matmul(
                        pmean[:], wsum_sl(s), Bt[:, ssl],
                        start=False, stop=(s == N_SUB - 1),
                    )
                st_mean[k] = pmean

            masks = []  # msb per image, filled by the mask prologue
            st_mf = {}

            def mfdma(k):
                """Mask reshape [nrows,128] -> [8,sub] (gpsimd SWDGE)."""
                b, row0, nrows = CHUNK_ROWS[k]
                sub = nrows * W // N_SUB
                mf = mfp.tile([N_SUB, sub], BF16, tag="mf", bufs=5,
                              padded_shape=[N_SUB, SUB])
                nc.gpsimd.dma_start(mf[:], masks[b][row0 : row0 + nrows, :])
                st_mf[k] = mf

            def frontB(k):
                """8-lane mask*mean -> ad (bf16 SBUF)."""
                b, row0, nrows = CHUNK_ROWS[k]
                sub = nrows * W // N_SUB
                pmean = st_mean[k]
                ad = adp.tile([N_SUB, sub], BF16, tag="ad",
                              padded_shape=[N_SUB, SUB])
                nc.vector.tensor_mul(ad[:], pmean[:], st_mf[k][:])
                st_ad[k] = ad

            def bcasts(k):
                """K=8 selector matmuls: corr quarters into psum (PE queue)."""
                b, row0, nrows = CHUNK_ROWS[k]
                sub = nrows * W // N_SUB
                ad = st_ad[k]
                pcs = []
                for q in range(4):
                    # each matmul output anchored at a PSUM bank boundary
                    # (offset 0 / SUB): a matmul may not cross a bank line
                    pc = pcp.tile([P, 2 * SUB], F32, tag="pc")
                    for h in range(2):
                        s = 2 * q + h
                        nc.tensor.matmul(
                            pc[:, h * SUB : h * SUB + sub],
                            sel8_sl(s), ad[:],
                            start=True, stop=True,
                        )
                    pcs.append(pc)
                return pcs

            def backend(k, pcs):
                """ACT evictions, bf16 adds, stores for chunk k."""
                b, csl, A, Bt = st_loads[k]
                _, row0, nrows = CHUNK_ROWS[k]
                ck = nrows * W
                sub = ck // N_SUB
                corr = corrp.tile([P, ck], BF16, tag="corr",
                                  padded_shape=[P, CHUNK])
                for q in range(4):
                    if sub == SUB:
                        nc.scalar.activation(
                            corr[:, q * 2 * sub : (q + 1) * 2 * sub],
                            pcs[q][:], mybir.ActivationFunctionType.Copy,
                        )
                    else:
                        for h in range(2):
                            nc.scalar.activation(
                                corr[:, (2 * q + h) * sub : (2 * q + h + 1) * sub],
                                pcs[q][:, h * SUB : h * SUB + sub],
                                mybir.ActivationFunctionType.Copy,
                            )
                half = ck // 2
                oA = outp.tile([P, ck], BF16, tag="oA", padded_shape=[P, CHUNK])
                oB = outp.tile([P, ck], BF16, tag="oB", padded_shape=[P, CHUNK])
                nc.vector.tensor_add(oA[:, 0:half], A[:, 0:half], corr[:, 0:half])
                nc.vector.tensor_add(oA[:, half:ck], A[:, half:ck], corr[:, half:ck])
                nc.scalar.dma_start(out[b, 0:P, csl], oA[:])
                nc.vector.tensor_add(oB[:, 0:half], Bt[:, 0:half], corr[:, 0:half])
                nc.vector.tensor_add(oB[:, half:ck], Bt[:, half:ck], corr[:, half:ck])
                nc.scalar.dma_start(out[b, P:C, csl], oB[:])

            # --- prologue: first chunks' loads right after the const DMA,
            # so the DMA stream starts the moment the preamble ends ---
            frontL(0)
            frontL(1)
            frontL(2)
            frontL(3)
            frontL(4)

            # --- per-image mask: psum[h,w] = sum_n rows[n,h]*cols[n,w]; clamp.
            # rows already carries validity & per-image selection (host). ---
            T3 = N_BOX_TILES
            for j in range(B_SHARD):
                pm = pcp.tile([P, 2 * SUB], F32, tag="pc")
                for t in range(T3):
                    nc.tensor.matmul(
                        pm[:, 0:W],
                        rows_sl(j, t),
                        cols_sl(t),
                        start=(t == 0), stop=(t == T3 - 1),
                    )
                msb = maskp.tile([P, W], BF16, tag=f"msb{j}")
                nc.vector.tensor_scalar_min(msb[:], pm[:, 0:W], 1.0)
                masks.append(msb)

            # --- software-pipelined main loop over the chunk schedule ---
            # (emission order = per-engine queue order: bcasts/backend of k
            # BEFORE next chunk's means/mul, so the store path never queues
            # behind future work)
            for k in range(4):
                mfdma(k)
            frontM(0)
            frontB(0)
            N = len(CHUNK_ROWS)
            for k in range(N):
                if k + 5 < N:
                    frontL(k + 5)
                if k + 4 < N:
                    mfdma(k + 4)
                if k + 1 < N:
                    frontM(k + 1)
                    frontB(k + 1)
                pcs = bcasts(k)
                backend(k, pcs)

    return nc


def _host_prep(x, bboxes, batch_idx):
    """Shard x; rasterize per-box row/col indicator tables (tiny host prep,
    mirrors the reference's int-trunc/clamp semantics exactly)."""
    x = np.ascontiguousarray(np.asarray(x, dtype=np.float32)).reshape(B, C, HW)
    x16 = x.astype(ml_dtypes.bfloat16)
    bb = np.asarray(bboxes, dtype=np.float32)
    bidx = np.asarray(batch_idx).astype(np.int64)

    xc, yc, bw, bh = bb[:, 0], bb[:, 1], bb[:, 2], bb[:, 3]
    x1 = np.maximum(0, np.trunc((xc - bw / 2) * W)).astype(np.int32)
    y1 = np.maximum(0, np.trunc((yc - bh / 2) * H)).astype(np.int32)
    x2 = np.minimum(W - 1, np.trunc((xc + bw / 2) * W)).astype(np.int32)
    y2 = np.minimum(H - 1, np.trunc((yc + bh / 2) * H)).astype(np.int32)
    valid = (x2 > x1) & (y2 > y1)

    hh = np.arange(H, dtype=np.int32)
    rows_ind = ((hh[None, :] >= y1[:, None]) & (hh[None, :] <= y2[:, None]))
    ww = np.arange(W, dtype=np.int32)
    cols_ind = ((ww[None, :] >= x1[:, None]) & (ww[None, :] <= x2[:, None]))

    R0 = B_SHARD * N_BOX_TILES * P  # 768
    C0 = R0 + N_BOX_TILES * P  # 1152
    W0 = C0 + N_SUB * N_SUB  # 1216
    CW = W0 + N_SUB * P  # 1728

    base = np.zeros((P, CW), dtype=ml_dtypes.bfloat16)
    # cols[n, t*128 + w]: box n = t*128 + p lives on partition p of tile t
    for t in range(N_BOX_TILES):
        lo, hi = t * P, min((t + 1) * P, N_BOX)
        base[0 : hi - lo, R0 + t * P : R0 + t * P + W] = cols_ind[lo:hi]
    # wsum: col 8s+j = ALPHA/C iff j==s  (mean lands on psum partition s)
    for s in range(N_SUB):
        base[:, C0 + N_SUB * s + s] = ALPHA / C
    # sel8 slice s = [8,128] stationary whose row s is all-ones
    for s in range(N_SUB):
        base[s, W0 + s * P : W0 + (s + 1) * P] = 1.0

    in_maps = []
    for i in range(N_CORES):
        cb = base.copy()
        for j in range(B_SHARD):
            on = valid & (bidx == (i * B_SHARD + j))
            for t in range(N_BOX_TILES):
                lo, hi = t * P, min((t + 1) * P, N_BOX)
                blk = rows_ind[lo:hi] * on[lo:hi, None]
                cb[0 : hi - lo, (j * N_BOX_TILES + t) * P : (j * N_BOX_TILES + t) * P + H] = blk
        in_maps.append(
            {
                "x": np.ascontiguousarray(x16[i * B_SHARD : (i + 1) * B_SHARD]),
                "consts": cb,
            }
        )
    return in_maps


def kernel(x, bboxes, batch_idx):
    in_maps = _host_prep(x, bboxes, batch_idx)
    nc = build_nc()
    nc.finalize()
    res = bass_utils.run_bass_kernel_spmd(nc, in_maps, core_ids=list(range(N_CORES)))
    shards = [res.results[i]["out"] for i in range(N_CORES)]
    return (
        np.concatenate(shards, axis=0).astype(np.float32).reshape(B, C, H, W)
    )


if __name__ == "__main__":
    nc = build_nc()
    nc.finalize()
    print("built ok:", len(nc.inst_map), "instructions")
